# revision 26
# baseline (speedup 1.0000x reference)
"""Trainium2 Bass kernel for cross-attention (cosine-normalized, 8 heads).

Reference computation (full inputs x,y [1,4096,64]):
  q = x@Wq+bq ; k,v = split(y@Wkv+bkv) ; per head (8 heads, dim 8):
  attn = softmax(l2norm(q) @ l2norm(k)^T) ; out = attn@v
  result = concat_heads(out) @ We + be
Sharding: one head per NeuronCore, host sums per-core resT partials.

v2 rewrite of the 217us baseline, targeting the exp floor:
  - Steady state is exp-bound on ScalarE (16.7M exps/core ~ 132us); the
    baseline wasted ~50us of prologue + ~25us tail around it.
  - bf16 everywhere on the PE (the baseline's f32r silently lowered to
    fp32_mode=HIGH at 4 cyc/row; bf16 is 1 cyc/row). Inputs ship bf16.
  - PE warmup matmuls at t=0 so projections run at 2.4 GHz, no gpsimd
    (its memsets + drain gated the old norm chain), DVE memsets instead.
  - Projections go through pipelined 2-bank PSUM pair tiles (pool
    rotation), q-path consumed by ACT (copy+square), k-path by DVE.
  - Selector matmuls at K=8 (no zero-padding of the squares tiles
    needed); sqrt reads selector PSUM directly; exp table preloaded via
    a dummy exp right after the sqrts.
  - Norm replication via row DMAs (sync=q, scalar=k queues), normalize
    muls in bf16 2x mode, ordered q0 / k-pairs first so the main loop
    starts ASAP.
  - Per-block epilogue fully overlapped under the next block's exp
    stream: den row DMA -> recip -> PE K=1 broadcast into the retired
    pos PSUM bank -> stage mul -> K=9 output projection at p64-127 of
    the same bank -> DVE copy -> sync DMA out. ScalarE does nothing but
    exp during the main loop.
"""

import sys

import numpy as np

for _p in ("/opt/trn_rl_repo",):
    if _p not in sys.path:
        sys.path.insert(0, _p)

from contextlib import ExitStack

import concourse.bass as bass
import concourse.tile as tile
from concourse import bacc, mybir
from concourse.bass import ts
from concourse.bass_utils import run_bass_kernel_spmd

F32 = mybir.dt.float32
BF16 = mybir.dt.bfloat16

HW = 4096          # sequence length
C = 64             # model dim
H = 8              # heads
D = 8              # head dim
CE = C + 1         # +ones row for bias folding
QB = 512           # q block
NQB = HW // QB     # 8
KC = 128           # k chunk
NKC = HW // KC     # 32
GROUPS = [3] * 10 + [2]   # k-chunks per exp/ACT group (32 total)
GMAX = max(GROUPS)
VW = D + 1         # v + ones column
NWARM = 12         # PE warmup matmuls

_BUILT = None
TRACE = False
LAST_RESULTS = None
DEBUG = False


def _body(ctx, tc, dram):
    nc = tc.nc
    xTe_d, yTe_d, wqe_d, wke_d, wve_d, webe_d, sel_d, out_d = dram[:8]

    const = ctx.enter_context(tc.tile_pool(name="const", bufs=1))
    expp = ctx.enter_context(tc.tile_pool(name="exps", bufs=4))
    ps_s = ctx.enter_context(tc.tile_pool(name="ps_s", bufs=2, space="PSUM"))
    ps_o = ctx.enter_context(tc.tile_pool(name="ps_o", bufs=2, space="PSUM"))

    # ---------------- SBUF tiles ----------------
    xTe = const.tile([KC, HW], BF16)     # rows 0..64 DMA'd, 65.. zeroed
    yTe = const.tile([KC, HW], BF16)
    qTn = const.tile([KC, HW], BF16)     # normalized q, rows 8.. zero
    kTn = const.tile([KC, HW], BF16)
    vext = const.tile([KC, VW * NKC], BF16)
    qT = const.tile([D, HW], BF16)       # raw q (transposed), bf16
    kT = const.tile([D, HW], BF16)
    sqq = const.tile([D, HW], BF16)      # squares (bf16, selector rhs)
    sqk = const.tile([D, HW], BF16)
    rep_q = const.tile([D, HW], BF16)    # inv norms replicated to D rows
    rep_k = const.tile([D, HW], BF16)
    oTe = const.tile([VW, HW], F32)      # numerator + den row
    stage = const.tile([VW, HW], BF16)   # normalized, den row == 1.0
    resT = const.tile([C, HW], F32)      # output staging
    warm_w = const.tile([KC, 16], BF16)
    sa_q = const.tile([D, QB], F32)      # sqrt of sum-squares
    sa_k = const.tile([D, QB], F32)
    inv_qf = const.tile([D, QB], F32)
    inv_kf = const.tile([D, QB], F32)
    inv_q = const.tile([D, QB], BF16)
    inv_k = const.tile([D, QB], BF16)
    scr_q = const.tile([D, QB], F32)
    scr_k = const.tile([D, QB], F32)
    warm = const.tile([1, 1], F32)

    # ---------------- t=0: small DVE memsets only -----------------------
    # (big zero-fills come from DRAM: a [128,4096] DVE memset is 3.5us
    # FD-serial; host ships padded xTe/yTe and a zeros tensor instead)
    U16 = mybir.dt.uint16
    warmz = const.tile([KC, QB], BF16)
    nc.vector.memset(warm_w[:].bitcast(U16), 0)
    nc.vector.memset(warmz[:].bitcast(U16), 0)
    nc.vector.memset(vext[:], 1.0)

    # ---------------- DMA loads (sync: q side, scalar: k/v side) --------
    wqe = const.tile([KC, D], BF16)
    wke = const.tile([KC, D], BF16)
    wve = const.tile([KC, D], BF16)
    webe = const.tile([VW, C], BF16)
    sel = const.tile([D, D * NQB], BF16)
    zz_d = dram[8]
    nc.sync.dma_start(wqe[:], wqe_d)
    nc.sync.dma_start(wke[:], wke_d)
    nc.scalar.dma_start(wve[:], wve_d)
    nc.scalar.dma_start(webe[:], webe_d)
    nc.scalar.dma_start(sel[:], sel_d)
    # x/y slabs interleaved across both queues so slab p of both tensors
    # lands early; zero-fills of qTn/kTn ride along afterwards.
    SLAB = HW // 4
    nc.sync.dma_start(xTe[:, ts(0, SLAB)], xTe_d[:, ts(0, SLAB)])
    nc.scalar.dma_start(yTe[:, ts(0, SLAB)], yTe_d[:, ts(0, SLAB)])
    nc.sync.dma_start(yTe[:, ts(1, SLAB)], yTe_d[:, ts(1, SLAB)])
    nc.scalar.dma_start(xTe[:, ts(1, SLAB)], xTe_d[:, ts(1, SLAB)])
    nc.sync.dma_start(xTe[:, ts(2, SLAB)], xTe_d[:, ts(2, SLAB)])
    nc.scalar.dma_start(yTe[:, ts(2, SLAB)], yTe_d[:, ts(2, SLAB)])
    nc.sync.dma_start(yTe[:, ts(3, SLAB)], yTe_d[:, ts(3, SLAB)])
    nc.scalar.dma_start(xTe[:, ts(3, SLAB)], xTe_d[:, ts(3, SLAB)])
    nc.sync.dma_start(kTn[:], zz_d)
    nc.scalar.dma_start(qTn[:], zz_d)

    # sqrt table preload (ScalarE, overlaps DMA wire time)
    nc.vector.memset(warm[:], 1.0)
    nc.scalar.sqrt(warm[:], warm[:])

    # ---------------- PE warmup (HAM to 8/8 before projections) ---------
    wps = ps_o.tile([KC, QB], F32, tag="pso", name="warmups")
    for i in range(NWARM):
        nc.tensor.matmul(wps[0:16, :], warm_w[:], warmz[:], start=True,
                         stop=True)

    # ---------------- projections (pipelined PSUM pairs at p0) ----------
    # pair p covers q/k blocks 2p,2p+1 -> [D, 2*QB] spanning 2 PSUM banks.
    # q pairs consumed by ACT (copy bf16 + square), k pairs by DVE.
    qps = [None] * 4
    kps = [None] * 4
    for p in range(4):
        qps[p] = ps_s.tile([D, 2 * QB], F32, tag="pss", name=f"qp{p}")
        for u in range(2):
            nc.tensor.matmul(qps[p][:, ts(u, QB)], wqe[:],
                             xTe[:, ts(2 * p + u, QB)], start=True, stop=True)
        kps[p] = ps_s.tile([D, 2 * QB], F32, tag="pss", name=f"kp{p}")
        for u in range(2):
            nc.tensor.matmul(kps[p][:, ts(u, QB)], wke[:],
                             yTe[:, ts(2 * p + u, QB)], start=True, stop=True)
    for p in range(4):
        sl = slice(2 * p * QB, (2 * p + 2) * QB)
        nc.scalar.copy(qT[:, sl], qps[p][:])
        nc.vector.tensor_copy(kT[:, sl], kps[p][:])
        nc.vector.tensor_mul(sqk[:, sl], kT[:, sl], kT[:, sl])
        nc.vector.tensor_mul(sqq[:, sl], qT[:, sl], qT[:, sl])

    # ---------------- v prep (row layout into ones-filled vext) ---------
    vps = ps_o.tile([KC, QB], F32, tag="pso", name="vprep")
    for c in range(NKC):
        nc.tensor.matmul(vps[:, c * D:(c + 1) * D], yTe[:, ts(c, KC)],
                         wve[:], start=True, stop=True)
    # one strided copy for all 32 chunks (skip the ones columns)
    vdst = vext[:].rearrange("p (c v) -> p c v", v=VW)[:, :, 0:D]
    vsrc = vps[:, 0:NKC * D].rearrange("p (c v) -> p c v", v=D)
    nc.vector.tensor_copy(vdst, vsrc)

    # ---------------- inverse norms -------------------------------------
    # selector matmuls K=8: one-hot col j sums squares into out row j.
    # q and k selectors get separate banks: the whole chain must stay at
    # partitions 0-7 (the flattening rep DMA misreads offset-32 sources).
    sps = ps_o.tile([D, QB], F32, tag="pso", name="selps")
    for j in range(NQB):
        nc.tensor.matmul(sps[:], sel[:, ts(j, D)], sqq[:, ts(j, QB)],
                         start=(j == 0), stop=(j == NQB - 1))
    sps_k = ps_o.tile([D, QB], F32, tag="pso", name="selpsk")
    for j in range(NQB):
        nc.tensor.matmul(sps_k[:], sel[:, ts(j, D)], sqk[:, ts(j, QB)],
                         start=(j == 0), stop=(j == NQB - 1))
    nc.scalar.sqrt(sa_q[:], sps[:])
    nc.scalar.sqrt(sa_k[:], sps_k[:])
    # preload the exp table now (ScalarE idle until the main loop)
    nc.scalar.activation(warm[:], warm[:], mybir.ActivationFunctionType.Exp)
    nc.vector.reciprocal_approx_accurate(inv_qf[:], sa_q[:], scr_q[:])
    nc.vector.tensor_copy(inv_q[:], inv_qf[:])
    nc.vector.reciprocal_approx_accurate(inv_kf[:], sa_k[:], scr_k[:])
    nc.vector.tensor_copy(inv_k[:], inv_kf[:])

    # replicate inv norms to D partitions (row DMAs, partition-crossing)
    for p in range(D):
        nc.sync.dma_start(rep_q[p:p + 1, :], inv_q[:])
        nc.scalar.dma_start(rep_k[p:p + 1, :], inv_k[:])

    # normalize muls (bf16 2x mode), ordered so the main loop unblocks
    # earliest: q block 0, then k pairs, interleaved with remaining q.
    def _qmul(j):
        nc.vector.tensor_mul(qTn[0:D, ts(j, QB)], qT[:, ts(j, QB)],
                             rep_q[:, ts(j, QB)])

    def _kmul(p):
        sl = slice(2 * p * QB, (2 * p + 2) * QB)
        nc.vector.tensor_mul(kTn[0:D, sl], kT[:, sl], rep_k[:, sl])

    _qmul(0)
    _kmul(0)
    _qmul(1)
    _kmul(1)
    _qmul(2)
    _kmul(2)
    _qmul(3)
    _kmul(3)
    for j in range(4, NQB):
        _qmul(j)

    # ---------------- main attention loop -------------------------------
    # oTe rows 0-7 numerator, row 8 softmax denominator. Flattened
    # (q-block, group) sequence with one group of mm1 lookahead.
    seq = []
    for j in range(NQB):
        c = 0
        for g in GROUPS:
            seq.append((j, c, g))
            c += g
    blk_start = {j: next(i for i, s in enumerate(seq) if s[0] == j)
                 for j in range(NQB)}
    pos = [None] * NQB
    pss = [None] * len(seq)

    def mm1(i):
        j, c, g = seq[i]
        ps = ps_s.tile([KC, GMAX * QB], F32, tag="pss", name=f"pss{i}")
        pss[i] = ps
        for u in range(g):
            nc.tensor.matmul(ps[:, ts(u, QB)], kTn[:, ts(c + u, KC)],
                             qTn[:, ts(j, QB)], start=True, stop=True)

    mm1(0)
    for i, (j, c, g) in enumerate(seq):
        if pos[j] is None:
            pos[j] = ps_o.tile([KC, QB], F32, tag="pso", name=f"po{j}")
        if i + 1 < len(seq):
            mm1(i + 1)
        ps = pss[i]
        es = expp.tile([KC, GMAX * QB], BF16, tag="es")
        nc.scalar.activation(es[:, 0:g * QB], ps[:, 0:g * QB],
                             mybir.ActivationFunctionType.Exp)
        for u in range(g):
            cc = c + u
            nc.tensor.matmul(pos[j][0:VW, :], vext[:, cc * VW:(cc + 1) * VW],
                             es[:, ts(u, QB)],
                             start=(cc == 0), stop=(cc == NKC - 1))
        pss[i] = None
        if c + g == NKC:
            nc.vector.tensor_copy(oTe[:, ts(j, QB)], pos[j][0:VW, :])

    # ---------------- normalize + output projection (sequential) --------
    den8 = const.tile([NQB, QB], F32)
    invd8 = const.tile([NQB, QB], F32)
    dscr8 = const.tile([NQB, QB], F32)
    rep9 = const.tile([VW, HW], F32)
    nc.sync.dma_start(den8[:], oTe[D:D + 1, :])
    nc.vector.reciprocal_approx_accurate(invd8[:], den8[:], dscr8[:])
    for p in range(VW):
        (nc.sync if p % 2 == 0 else nc.scalar).dma_start(
            rep9[p:p + 1, :], invd8[:])
    for j in range(NQB):
        nc.vector.tensor_mul(stage[:, ts(j, QB)], oTe[:, ts(j, QB)],
                             rep9[:, ts(j, QB)])
        psj = ps_s.tile([C, QB], F32, tag="pss", name=f"op{j}")
        nc.tensor.matmul(psj[:], webe[:], stage[:, ts(j, QB)], start=True,
                         stop=True)
        nc.vector.tensor_copy(resT[0:C, ts(j, QB)], psj[:])
        nc.sync.dma_start(out_d[:, ts(j, QB)], resT[0:C, ts(j, QB)])

    if DEBUG:
        dbg = dram[9]
        nc.sync.dma_start(dbg["qTn"], qTn[0:D, :])
        nc.sync.dma_start(dbg["kTn"], kTn[0:D, :])
        nc.sync.dma_start(dbg["vext"], vext[:])
        nc.sync.dma_start(dbg["oTe"], oTe[:])
        nc.sync.dma_start(dbg["repq"], rep_q[:])
        nc.sync.dma_start(dbg["repk"], rep_k[:])
        nc.sync.dma_start(dbg["sqq"], sqq[:])
        nc.sync.dma_start(dbg["qT"], qT[:])
        nc.sync.dma_start(dbg["kT"], kT[:])


def _build():
    global _BUILT
    if _BUILT is not None:
        return _BUILT
    nc = bacc.Bacc("TRN2", target_bir_lowering=False, debug=False, num_devices=H)
    xTe_d = nc.dram_tensor("xTe", [KC, HW], BF16, kind="ExternalInput").ap()
    yTe_d = nc.dram_tensor("yTe", [KC, HW], BF16, kind="ExternalInput").ap()
    wqe_d = nc.dram_tensor("wqe", [KC, D], BF16, kind="ExternalInput").ap()
    wke_d = nc.dram_tensor("wke", [KC, D], BF16, kind="ExternalInput").ap()
    wve_d = nc.dram_tensor("wve", [KC, D], BF16, kind="ExternalInput").ap()
    webe_d = nc.dram_tensor("webe", [VW, C], BF16, kind="ExternalInput").ap()
    sel_d = nc.dram_tensor("sel", [D, D * NQB], BF16, kind="ExternalInput").ap()
    zz_d = nc.dram_tensor("zz", [KC, HW], BF16, kind="ExternalInput").ap()
    out_d = nc.dram_tensor("resT", [C, HW], F32, kind="ExternalOutput").ap()
    dbg = None
    if DEBUG:
        dbg = {
            "qTn": nc.dram_tensor("d_qTn", [D, HW], BF16, kind="ExternalOutput").ap(),
            "kTn": nc.dram_tensor("d_kTn", [D, HW], BF16, kind="ExternalOutput").ap(),
            "vext": nc.dram_tensor("d_vext", [KC, VW * NKC], BF16, kind="ExternalOutput").ap(),
            "oTe": nc.dram_tensor("d_oTe", [VW, HW], F32, kind="ExternalOutput").ap(),
            "repq": nc.dram_tensor("d_repq", [D, HW], BF16, kind="ExternalOutput").ap(),
            "repk": nc.dram_tensor("d_repk", [D, HW], BF16, kind="ExternalOutput").ap(),
            "sqq": nc.dram_tensor("d_sqq", [D, HW], BF16, kind="ExternalOutput").ap(),
            "qT": nc.dram_tensor("d_qT", [D, HW], BF16, kind="ExternalOutput").ap(),
            "kT": nc.dram_tensor("d_kT", [D, HW], BF16, kind="ExternalOutput").ap(),
        }
    with tile.TileContext(nc) as tc, ExitStack() as ctx:
        _body(ctx, tc, (xTe_d, yTe_d, wqe_d, wke_d, wve_d, webe_d, sel_d,
                        out_d[:], zz_d, dbg))
    nc.compile()
    _BUILT = nc
    return nc


def make_in_maps(x, y, Wq, bq, Wkv, bkv, We, be):
    import ml_dtypes
    bf16 = ml_dtypes.bfloat16
    x, y, Wq, bq, Wkv, bkv, We, be = (
        np.asarray(a, np.float32) for a in (x, y, Wq, bq, Wkv, bkv, We, be))
    ones = np.ones((1, HW), np.float32)
    zrows = np.zeros((KC - CE, HW), np.float32)
    xTe = np.ascontiguousarray(np.vstack([x[0].T, ones, zrows])).astype(bf16)
    yTe = np.ascontiguousarray(np.vstack([y[0].T, ones, zrows])).astype(bf16)
    zz = np.zeros((KC, HW), np.float32).astype(bf16)
    sel = np.zeros((D, D * NQB), np.float32)
    for j in range(NQB):
        sel[0:D, D * j + j] = 1.0
    sel = sel.astype(bf16)
    zpad = np.zeros((KC - CE, D), np.float32)
    in_maps = []
    for h in range(H):
        sl = slice(h * D, (h + 1) * D)
        slv = slice(C + h * D, C + (h + 1) * D)
        in_maps.append({
            "xTe": xTe,
            "yTe": yTe,
            "wqe": np.ascontiguousarray(
                np.vstack([Wq[:, sl], bq[None, sl], zpad])).astype(bf16),
            "wke": np.ascontiguousarray(
                np.vstack([Wkv[:, sl], bkv[None, sl], zpad])).astype(bf16),
            "wve": np.ascontiguousarray(
                np.vstack([Wkv[:, slv], bkv[None, slv], zpad])).astype(bf16),
            "webe": np.ascontiguousarray(
                np.vstack([We[sl, :], be[None, :] / H])).astype(bf16),
            "sel": sel,
            "zz": zz,
        })
    return in_maps


def kernel(x, y, Wq, bq, Wkv, bkv, We, be):
    global LAST_RESULTS
    nc = _build()
    in_maps = make_in_maps(x, y, Wq, bq, Wkv, bkv, We, be)
    res = run_bass_kernel_spmd(nc, in_maps, core_ids=list(range(H)), trace=TRACE)
    LAST_RESULTS = res
    acc = np.zeros((C, HW), np.float64)
    for r in res.results:
        acc += r["resT"]
    return np.ascontiguousarray(acc.T[None]).astype(np.float32)


# revision 37
# speedup vs baseline: 1.1285x; 1.1285x over previous
"""Trainium2 Bass kernel for cross-attention (cosine-normalized, 8 heads).

Reference computation (full inputs x,y [1,4096,64]):
  q = x@Wq+bq ; k,v = split(y@Wkv+bkv) ; per head (8 heads, dim 8):
  attn = softmax(l2norm(q) @ l2norm(k)^T) ; out = attn@v
  result = concat_heads(out) @ We + be
Sharding: one head per NeuronCore, host sums per-core resT partials.

v2 rewrite of the 217us baseline, targeting the exp floor:
  - Steady state is exp-bound on ScalarE (16.7M exps/core ~ 132us); the
    baseline wasted ~50us of prologue + ~25us tail around it.
  - bf16 everywhere on the PE (the baseline's f32r silently lowered to
    fp32_mode=HIGH at 4 cyc/row; bf16 is 1 cyc/row). Inputs ship bf16.
  - PE warmup matmuls at t=0 so projections run at 2.4 GHz, no gpsimd
    (its memsets + drain gated the old norm chain), DVE memsets instead.
  - Projections go through pipelined 2-bank PSUM pair tiles (pool
    rotation), q-path consumed by ACT (copy+square), k-path by DVE.
  - Selector matmuls at K=8 (no zero-padding of the squares tiles
    needed); sqrt reads selector PSUM directly; exp table preloaded via
    a dummy exp right after the sqrts.
  - Norm replication via row DMAs (sync=q, scalar=k queues), normalize
    muls in bf16 2x mode, ordered q0 / k-pairs first so the main loop
    starts ASAP.
  - Per-block epilogue fully overlapped under the next block's exp
    stream: den row DMA -> recip -> PE K=1 broadcast into the retired
    pos PSUM bank -> stage mul -> K=9 output projection at p64-127 of
    the same bank -> DVE copy -> sync DMA out. ScalarE does nothing but
    exp during the main loop.
"""

import sys

import numpy as np

for _p in ("/opt/trn_rl_repo",):
    if _p not in sys.path:
        sys.path.insert(0, _p)

from contextlib import ExitStack

import concourse.bass as bass
import concourse.tile as tile
from concourse import bacc, mybir
from concourse.bass import ts
from concourse.bass_utils import run_bass_kernel_spmd

F32 = mybir.dt.float32
BF16 = mybir.dt.bfloat16

HW = 4096          # sequence length
C = 64             # model dim
H = 8              # heads
D = 8              # head dim
CE = C + 1         # +ones row for bias folding
QB = 512           # q block
NQB = HW // QB     # 8
KC = 128           # k chunk
NKC = HW // KC     # 32
GROUPS = [3] * 10 + [2]   # k-chunks per exp/ACT group (32 total)
GMAX = max(GROUPS)
VW = D + 1         # v + ones column
NWARM = 12         # PE warmup matmuls

_BUILT = None
TRACE = False
LAST_RESULTS = None
DEBUG = False


def _body(ctx, tc, dram):
    nc = tc.nc
    xTe_d, yTe_d, wqe_d, wke_d, wve_d, webe_d, sel_d, out_d = dram[:8]

    const = ctx.enter_context(tc.tile_pool(name="const", bufs=1))
    expp = ctx.enter_context(tc.tile_pool(name="exps", bufs=4))
    ps_s = ctx.enter_context(tc.tile_pool(name="ps_s", bufs=2, space="PSUM"))
    ps_o = ctx.enter_context(tc.tile_pool(name="ps_o", bufs=2, space="PSUM"))

    # ---------------- SBUF tiles ----------------
    xTe = const.tile([KC, HW], BF16)     # rows 0..64 DMA'd, 65.. zeroed
    yTe = const.tile([KC, HW], BF16)
    qTn = const.tile([KC, HW], BF16)     # normalized q, rows 8.. zero
    kTn = const.tile([KC, HW], BF16)
    vext = const.tile([KC, VW * NKC], BF16)
    qT = const.tile([D, HW], BF16)       # raw q (transposed), bf16
    kT = const.tile([D, HW], BF16)
    sqq = const.tile([D, HW], BF16)      # squares (bf16, selector rhs)
    sqk = const.tile([D, HW], BF16)
    rep_q = const.tile([D, HW], BF16)    # inv norms replicated to D rows
    rep_k = const.tile([D, HW], BF16)
    oTe = const.tile([VW, HW], F32)      # numerator + den row
    stage = const.tile([VW, HW], BF16)   # normalized, den row == 1.0
    resT = const.tile([KC, HW], F32)     # output staging (rows 64..127)
    dn = const.tile([1, QB], F32)        # per-block den row at p0
    invd_f = const.tile([1, QB], F32)
    invd = const.tile([1, QB], BF16)
    dscr = const.tile([1, QB], F32)
    ones9 = const.tile([1, 16], BF16)
    warm_w = const.tile([KC, 16], BF16)
    sa_q = const.tile([D, QB], F32)      # sqrt of sum-squares
    sa_k = const.tile([D, QB], F32)
    inv_qf = const.tile([D, QB], F32)
    inv_kf = const.tile([D, QB], F32)
    inv_q = const.tile([D, QB], BF16)
    inv_k = const.tile([D, QB], BF16)
    scr_q = const.tile([D, QB], F32)
    scr_k = const.tile([D, QB], F32)
    warm = const.tile([1, 1], F32)

    # ---------------- t=0: small DVE memsets only -----------------------
    # (big zero-fills come from DRAM: a [128,4096] DVE memset is 3.5us
    # FD-serial; host ships padded xTe/yTe and a zeros tensor instead)
    U16 = mybir.dt.uint16
    warmz = const.tile([KC, QB], BF16)
    nc.vector.memset(warm_w[:].bitcast(U16), 0)
    nc.vector.memset(warmz[:].bitcast(U16), 0)
    nc.vector.memset(vext[:], 1.0)
    nc.vector.memset(ones9[:], 1.0)

    # ---------------- DMA loads --------------------------------------
    # q/k/v weights packed in ONE [128, 24] tensor: a [128, 8] DMA costs
    # 128 tiny descriptors; three of them serialized starved the queues.
    wq3 = const.tile([KC, 3 * D], BF16)
    wqe = wq3[:, 0:D]
    wke = wq3[:, D:2 * D]
    wve = wq3[:, 2 * D:3 * D]
    webe = const.tile([VW, C], BF16)
    sel = const.tile([D, D * NQB], BF16)
    zz_d, wq3_d = dram[8], dram[9]  # noqa: wq/wk/wv packed
    nc.sync.dma_start(wq3[:], wq3_d)
    nc.scalar.dma_start(webe[:], webe_d)
    nc.scalar.dma_start(sel[:], sel_d)
    # x/y slabs interleaved across both queues so slab p of both tensors
    # lands early; zero-fills of qTn/kTn ride along afterwards.
    SLAB = HW // 4
    nc.sync.dma_start(xTe[:, ts(0, SLAB)], xTe_d[:, ts(0, SLAB)])
    nc.scalar.dma_start(yTe[:, ts(0, SLAB)], yTe_d[:, ts(0, SLAB)])
    nc.sync.dma_start(yTe[:, ts(1, SLAB)], yTe_d[:, ts(1, SLAB)])
    nc.scalar.dma_start(xTe[:, ts(1, SLAB)], xTe_d[:, ts(1, SLAB)])
    nc.sync.dma_start(xTe[:, ts(2, SLAB)], xTe_d[:, ts(2, SLAB)])
    nc.scalar.dma_start(yTe[:, ts(2, SLAB)], yTe_d[:, ts(2, SLAB)])
    nc.sync.dma_start(yTe[:, ts(3, SLAB)], yTe_d[:, ts(3, SLAB)])
    nc.scalar.dma_start(xTe[:, ts(3, SLAB)], xTe_d[:, ts(3, SLAB)])
    nc.sync.dma_start(kTn[:], zz_d)
    nc.scalar.dma_start(qTn[:], zz_d)

    # sqrt table preload (ScalarE, overlaps DMA wire time)
    nc.vector.memset(warm[:], 1.0)
    nc.scalar.sqrt(warm[:], warm[:])

    # ---------------- PE warmup (HAM to 8/8 before projections) ---------
    wps = ps_o.tile([KC, QB], F32, tag="pso", name="warmups")
    for i in range(NWARM):
        nc.tensor.matmul(wps[0:16, :], warm_w[:], warmz[:], start=True,
                         stop=True)

    # ---------------- projections (pipelined PSUM pairs at p0) ----------
    # pair p covers q/k blocks 2p,2p+1 -> [D, 2*QB] spanning 2 PSUM banks.
    # q pairs consumed by ACT (copy bf16 + square), k pairs by DVE.
    qps = [None] * 4
    kps = [None] * 4
    for p in range(4):
        qps[p] = ps_s.tile([D, 2 * QB], F32, tag="pss", name=f"qp{p}")
        for u in range(2):
            nc.tensor.matmul(qps[p][:, ts(u, QB)], wqe,
                             xTe[:, ts(2 * p + u, QB)], start=True, stop=True)
        kps[p] = ps_s.tile([D, 2 * QB], F32, tag="pss", name=f"kp{p}")
        for u in range(2):
            nc.tensor.matmul(kps[p][:, ts(u, QB)], wke,
                             yTe[:, ts(2 * p + u, QB)], start=True, stop=True)
        if p < 2:
            # keeper matmuls: bridge DMA waits so HAM never re-throttles
            for _ in range(3):
                nc.tensor.matmul(wps[0:16, :], warm_w[:], warmz[:],
                                 start=True, stop=True)
    for p in range(4):
        sl = slice(2 * p * QB, (2 * p + 2) * QB)
        nc.scalar.copy(qT[:, sl], qps[p][:])
        nc.vector.tensor_copy(kT[:, sl], kps[p][:])
        nc.vector.tensor_mul(sqk[:, sl], kT[:, sl], kT[:, sl])
        nc.vector.tensor_mul(sqq[:, sl], qT[:, sl], qT[:, sl])

    # ---------------- v prep (row layout into ones-filled vext) ---------
    vps = ps_o.tile([KC, QB], F32, tag="pso", name="vprep")
    for c in range(NKC):
        nc.tensor.matmul(vps[:, c * D:(c + 1) * D], yTe[:, ts(c, KC)],
                         wve, start=True, stop=True)
    # one strided copy for all 32 chunks (skip the ones columns)
    vdst = vext[:].rearrange("p (c v) -> p c v", v=VW)[:, :, 0:D]
    vsrc = vps[:, 0:NKC * D].rearrange("p (c v) -> p c v", v=D)
    nc.vector.tensor_copy(vdst, vsrc)

    # ---------------- inverse norms -------------------------------------
    # selector matmuls K=8: one-hot col j sums squares into out row j.
    # q and k selectors get separate banks: the whole chain must stay at
    # partitions 0-7 (the flattening rep DMA misreads offset-32 sources).
    sps = ps_o.tile([D, QB], F32, tag="pso", name="selps")
    for j in range(NQB):
        nc.tensor.matmul(sps[:], sel[:, ts(j, D)], sqq[:, ts(j, QB)],
                         start=(j == 0), stop=(j == NQB - 1))
    sps_k = ps_o.tile([D, QB], F32, tag="pso", name="selpsk")
    for j in range(NQB):
        nc.tensor.matmul(sps_k[:], sel[:, ts(j, D)], sqk[:, ts(j, QB)],
                         start=(j == 0), stop=(j == NQB - 1))
    nc.scalar.sqrt(sa_q[:], sps[:])
    nc.scalar.sqrt(sa_k[:], sps_k[:])
    # preload the exp table now; reading sa_k pins this AFTER the sqrts so
    # the scheduler cannot hoist it (each hoist costs 2 extra table loads)
    nc.scalar.activation(warm[:], sa_k[0:1, 0:1],
                         mybir.ActivationFunctionType.Exp)
    nc.vector.reciprocal_approx_accurate(inv_kf[:], sa_k[:], scr_k[:])
    nc.vector.tensor_copy(inv_k[:], inv_kf[:])
    nc.vector.reciprocal_approx_accurate(inv_qf[:], sa_q[:], scr_q[:])
    nc.vector.tensor_copy(inv_q[:], inv_qf[:])

    # replicate inv norms to D partitions (row DMAs, partition-crossing);
    # k rows first on BOTH queues: the exp stream consumes k blocks fast
    for p in range(D):
        (nc.sync if p % 2 == 0 else nc.scalar).dma_start(
            rep_k[p:p + 1, :], inv_k[:])
    for p in range(D):
        (nc.sync if p % 2 == 0 else nc.scalar).dma_start(
            rep_q[p:p + 1, :], inv_q[:])

    # normalize muls (bf16 2x mode), per 512-block, consumption-ordered
    def _qmul(j):
        nc.vector.tensor_mul(qTn[0:D, ts(j, QB)], qT[:, ts(j, QB)],
                             rep_q[:, ts(j, QB)])

    def _kmul(j):
        nc.vector.tensor_mul(kTn[0:D, ts(j, QB)], kT[:, ts(j, QB)],
                             rep_k[:, ts(j, QB)])

    _kmul(0)
    _qmul(0)
    for j in range(1, NQB):
        _kmul(j)
    for j in range(1, NQB):
        _qmul(j)

    # ---------------- main attention loop -------------------------------
    # oTe rows 0-7 numerator, row 8 softmax denominator. Flattened
    # (q-block, group) sequence with one group of mm1 lookahead.
    seq = []
    for j in range(NQB):
        c = 0
        for g in GROUPS:
            seq.append((j, c, g))
            c += g
    blk_start = {j: next(i for i, s in enumerate(seq) if s[0] == j)
                 for j in range(NQB)}
    pos = [None] * NQB
    pss = [None] * len(seq)

    def mm1(i):
        j, c, g = seq[i]
        ps = ps_s.tile([KC, GMAX * QB], F32, tag="pss", name=f"pss{i}")
        pss[i] = ps
        for u in range(g):
            nc.tensor.matmul(ps[:, ts(u, QB)], kTn[:, ts(c + u, KC)],
                             qTn[:, ts(j, QB)], start=True, stop=True)

    es_t = [None] * len(seq)

    def mm2(i):
        j, c, g = seq[i]
        if pos[j] is None:
            pos[j] = ps_o.tile([KC, QB], F32, tag="pso", name=f"po{j}")
        es = es_t[i]
        for u in range(g):
            cc = c + u
            nc.tensor.matmul(pos[j][0:VW, :], vext[:, cc * VW:(cc + 1) * VW],
                             es[:, ts(u, QB)],
                             start=(cc == 0), stop=(cc == NKC - 1))
        es_t[i] = None
        if c + g == NKC:
            epi_a(j)

    # per-block epilogue, staged under the next block's groups; everything
    # lives at partitions 0-8 / 64-127 of the retired pos bank
    def epi_a(j):      # right after last mm2 of block j
        nc.vector.tensor_copy(oTe[:, ts(j, QB)], pos[j][0:VW, :])
        nc.sync.dma_start(dn[:], oTe[D:D + 1, ts(j, QB)])
        nc.vector.reciprocal_approx_accurate(invd_f[:], dn[:], dscr[:])
        nc.vector.tensor_copy(invd[:], invd_f[:])

    def epi_b(j):      # PE K=1 broadcast of 1/den into pos[j] rows 0-8
        nc.tensor.matmul(pos[j][0:VW, :], ones9[:, 0:VW], invd[:],
                         start=True, stop=True)

    def epi_c(j):      # stage mul + K=9 output projection + copy + DMA out
        nc.vector.tensor_mul(stage[:, ts(j, QB)], oTe[:, ts(j, QB)],
                             pos[j][0:VW, :])
        nc.tensor.matmul(pos[j][64:64 + C, :], webe[:],
                         stage[:, ts(j, QB)], start=True, stop=True)
        nc.vector.tensor_copy(resT[64:64 + C, ts(j, QB)],
                              pos[j][64:64 + C, :])
        nc.sync.dma_start(out_d[:, ts(j, QB)], resT[64:64 + C, ts(j, QB)])

    pending = {}

    # software pipeline with mm2 deferred one group: while exp(i) runs on
    # ScalarE, the PE executes mm1(i+1) then mm2(i-1) -- so exp(i+1) never
    # sits behind an mm2 that itself waits on exp(i) (head-of-line stall)
    mm1(0)
    for i, (j, c, g) in enumerate(seq):
        for fn in pending.pop(i, ()):
            fn()
        if i + 1 < len(seq):
            mm1(i + 1)
        es = expp.tile([KC, GMAX * QB], BF16, tag="es")
        es_t[i] = es
        nc.scalar.activation(es[:, 0:g * QB], pss[i][:, 0:g * QB],
                             mybir.ActivationFunctionType.Exp)
        pss[i] = None
        if i > 0:
            mm2(i - 1)
        if c + g == NKC and j + 1 < NQB:
            s = blk_start[j + 1]
            pending.setdefault(s + 3, []).append(lambda j=j: epi_b(j))
            pending.setdefault(s + 5, []).append(lambda j=j: epi_c(j))
    mm2(len(seq) - 1)
    epi_b(NQB - 1)
    epi_c(NQB - 1)

    if DEBUG:
        dbg = dram[10]
        nc.sync.dma_start(dbg["qTn"], qTn[0:D, :])
        nc.sync.dma_start(dbg["kTn"], kTn[0:D, :])
        nc.sync.dma_start(dbg["vext"], vext[:])
        nc.sync.dma_start(dbg["oTe"], oTe[:])
        nc.sync.dma_start(dbg["repq"], rep_q[:])
        nc.sync.dma_start(dbg["repk"], rep_k[:])
        nc.sync.dma_start(dbg["sqq"], sqq[:])
        nc.sync.dma_start(dbg["qT"], qT[:])
        nc.sync.dma_start(dbg["kT"], kT[:])


def _build():
    global _BUILT
    if _BUILT is not None:
        return _BUILT
    nc = bacc.Bacc("TRN2", target_bir_lowering=False, debug=False, num_devices=H)
    xTe_d = nc.dram_tensor("xTe", [KC, HW], BF16, kind="ExternalInput").ap()
    yTe_d = nc.dram_tensor("yTe", [KC, HW], BF16, kind="ExternalInput").ap()
    wq3_d = nc.dram_tensor("wq3", [KC, 3 * D], BF16, kind="ExternalInput").ap()
    webe_d = nc.dram_tensor("webe", [VW, C], BF16, kind="ExternalInput").ap()
    sel_d = nc.dram_tensor("sel", [D, D * NQB], BF16, kind="ExternalInput").ap()
    zz_d = nc.dram_tensor("zz", [KC, HW], BF16, kind="ExternalInput").ap()
    out_d = nc.dram_tensor("resT", [C, HW], F32, kind="ExternalOutput").ap()
    dbg = None
    if DEBUG:
        dbg = {
            "qTn": nc.dram_tensor("d_qTn", [D, HW], BF16, kind="ExternalOutput").ap(),
            "kTn": nc.dram_tensor("d_kTn", [D, HW], BF16, kind="ExternalOutput").ap(),
            "vext": nc.dram_tensor("d_vext", [KC, VW * NKC], BF16, kind="ExternalOutput").ap(),
            "oTe": nc.dram_tensor("d_oTe", [VW, HW], F32, kind="ExternalOutput").ap(),
            "repq": nc.dram_tensor("d_repq", [D, HW], BF16, kind="ExternalOutput").ap(),
            "repk": nc.dram_tensor("d_repk", [D, HW], BF16, kind="ExternalOutput").ap(),
            "sqq": nc.dram_tensor("d_sqq", [D, HW], BF16, kind="ExternalOutput").ap(),
            "qT": nc.dram_tensor("d_qT", [D, HW], BF16, kind="ExternalOutput").ap(),
            "kT": nc.dram_tensor("d_kT", [D, HW], BF16, kind="ExternalOutput").ap(),
        }
    with tile.TileContext(nc) as tc, ExitStack() as ctx:
        _body(ctx, tc, (xTe_d, yTe_d, None, None, None, webe_d, sel_d,
                        out_d[:], zz_d, wq3_d, dbg))
    nc.compile()
    _BUILT = nc
    return nc


def make_in_maps(x, y, Wq, bq, Wkv, bkv, We, be):
    import ml_dtypes
    bf16 = ml_dtypes.bfloat16
    x, y, Wq, bq, Wkv, bkv, We, be = (
        np.asarray(a, np.float32) for a in (x, y, Wq, bq, Wkv, bkv, We, be))
    ones = np.ones((1, HW), np.float32)
    zrows = np.zeros((KC - CE, HW), np.float32)
    xTe = np.ascontiguousarray(np.vstack([x[0].T, ones, zrows])).astype(bf16)
    yTe = np.ascontiguousarray(np.vstack([y[0].T, ones, zrows])).astype(bf16)
    zz = np.zeros((KC, HW), np.float32).astype(bf16)
    sel = np.zeros((D, D * NQB), np.float32)
    for j in range(NQB):
        sel[0:D, D * j + j] = 1.0
    sel = sel.astype(bf16)
    zpad = np.zeros((KC - CE, D), np.float32)
    in_maps = []
    for h in range(H):
        sl = slice(h * D, (h + 1) * D)
        slv = slice(C + h * D, C + (h + 1) * D)
        wqe = np.vstack([Wq[:, sl], bq[None, sl], zpad])
        wke = np.vstack([Wkv[:, sl], bkv[None, sl], zpad])
        wve = np.vstack([Wkv[:, slv], bkv[None, slv], zpad])
        in_maps.append({
            "xTe": xTe,
            "yTe": yTe,
            "wq3": np.ascontiguousarray(
                np.hstack([wqe, wke, wve])).astype(bf16),
            "webe": np.ascontiguousarray(
                np.vstack([We[sl, :], be[None, :] / H])).astype(bf16),
            "sel": sel,
            "zz": zz,
        })
    return in_maps


def kernel(x, y, Wq, bq, Wkv, bkv, We, be):
    global LAST_RESULTS
    nc = _build()
    in_maps = make_in_maps(x, y, Wq, bq, Wkv, bkv, We, be)
    res = run_bass_kernel_spmd(nc, in_maps, core_ids=list(range(H)), trace=TRACE)
    LAST_RESULTS = res
    acc = np.zeros((C, HW), np.float64)
    for r in res.results:
        acc += r["resT"]
    return np.ascontiguousarray(acc.T[None]).astype(np.float32)


# revision 43
# speedup vs baseline: 1.1725x; 1.0390x over previous
"""Trainium2 Bass kernel for cross-attention (cosine-normalized, 8 heads).

Reference computation (full inputs x,y [1,4096,64]):
  q = x@Wq+bq ; k,v = split(y@Wkv+bkv) ; per head (8 heads, dim 8):
  attn = softmax(l2norm(q) @ l2norm(k)^T) ; out = attn@v
  result = concat_heads(out) @ We + be
Sharding: one head per NeuronCore, host sums per-core resT partials.

v2 rewrite of the 217us baseline, targeting the exp floor:
  - Steady state is exp-bound on ScalarE (16.7M exps/core ~ 132us); the
    baseline wasted ~50us of prologue + ~25us tail around it.
  - bf16 everywhere on the PE (the baseline's f32r silently lowered to
    fp32_mode=HIGH at 4 cyc/row; bf16 is 1 cyc/row). Inputs ship bf16.
  - PE warmup matmuls at t=0 so projections run at 2.4 GHz, no gpsimd
    (its memsets + drain gated the old norm chain), DVE memsets instead.
  - Projections go through pipelined 2-bank PSUM pair tiles (pool
    rotation), q-path consumed by ACT (copy+square), k-path by DVE.
  - Selector matmuls at K=8 (no zero-padding of the squares tiles
    needed); sqrt reads selector PSUM directly; exp table preloaded via
    a dummy exp right after the sqrts.
  - Norm replication via row DMAs (sync=q, scalar=k queues), normalize
    muls in bf16 2x mode, ordered q0 / k-pairs first so the main loop
    starts ASAP.
  - Per-block epilogue fully overlapped under the next block's exp
    stream: den row DMA -> recip -> PE K=1 broadcast into the retired
    pos PSUM bank -> stage mul -> K=9 output projection at p64-127 of
    the same bank -> DVE copy -> sync DMA out. ScalarE does nothing but
    exp during the main loop.
"""

import sys

import numpy as np

for _p in ("/opt/trn_rl_repo",):
    if _p not in sys.path:
        sys.path.insert(0, _p)

from contextlib import ExitStack

import concourse.bass as bass
import concourse.tile as tile
from concourse import bacc, mybir
from concourse.bass import ts
from concourse.bass_utils import run_bass_kernel_spmd

F32 = mybir.dt.float32
BF16 = mybir.dt.bfloat16

HW = 4096          # sequence length
C = 64             # model dim
H = 8              # heads
D = 8              # head dim
CE = C + 1         # +ones row for bias folding
QB = 512           # q block
NQB = HW // QB     # 8
KC = 128           # k chunk
NKC = HW // KC     # 32
GROUPS = [3] * 10 + [2]   # k-chunks per exp/ACT group (32 total)
GMAX = max(GROUPS)
VW = D + 1         # v + ones column
NWARM = 12         # PE warmup matmuls

_BUILT = None
TRACE = False
LAST_RESULTS = None
DEBUG = False


def _body(ctx, tc, dram):
    nc = tc.nc
    xTe_d, yTe_d, wqe_d, wke_d, wve_d, webe_d, sel_d, out_d = dram[:8]

    const = ctx.enter_context(tc.tile_pool(name="const", bufs=1))
    expp = ctx.enter_context(tc.tile_pool(name="exps", bufs=4))
    ps_s = ctx.enter_context(tc.tile_pool(name="ps_s", bufs=2, space="PSUM"))
    ps_o = ctx.enter_context(tc.tile_pool(name="ps_o", bufs=2, space="PSUM"))

    # ---------------- SBUF tiles ----------------
    xTe = const.tile([KC, HW], BF16)     # rows 0..64 DMA'd, 65.. zeroed
    yTe = const.tile([KC, HW], BF16)
    qTn = const.tile([KC, HW], BF16)     # normalized q, rows 8.. zero
    kTn = const.tile([KC, HW], BF16)
    vext = const.tile([KC, VW * NKC], BF16)
    qT = const.tile([D, HW], BF16)       # raw q (transposed), bf16
    kT = const.tile([D, HW], BF16)
    sqq = const.tile([D, HW], BF16)      # squares (bf16, selector rhs)
    sqk = const.tile([D, HW], BF16)
    rep_q = const.tile([D, HW], BF16)    # inv norms replicated to D rows
    rep_k = const.tile([D, HW], BF16)
    oTe = const.tile([VW, HW], F32)      # den row 0 + numerator rows 1-8
    stage = const.tile([VW, HW], BF16)   # normalized, den row == 1.0
    resT = const.tile([KC, HW], F32)     # output staging (rows 64..127)
    invd_f = const.tile([1, QB], F32)
    invd = const.tile([1, QB], BF16)
    dscr = const.tile([1, QB], F32)
    ones9 = const.tile([1, 16], BF16)
    warm_w = const.tile([KC, 16], BF16)
    sa_q = const.tile([D, QB], F32)      # sqrt of sum-squares
    sa_k = const.tile([D, QB], F32)
    inv_qf = const.tile([D, QB], F32)
    inv_kf = const.tile([D, QB], F32)
    inv_q = const.tile([D, QB], BF16)
    inv_k = const.tile([D, QB], BF16)
    scr_q = const.tile([D, QB], F32)
    scr_k = const.tile([D, QB], F32)
    warm = const.tile([1, 1], F32)

    # ---------------- t=0: small DVE memsets only -----------------------
    # (big zero-fills come from DRAM: a [128,4096] DVE memset is 3.5us
    # FD-serial; host ships padded xTe/yTe and a zeros tensor instead)
    U16 = mybir.dt.uint16
    warmz = const.tile([KC, QB], BF16)
    nc.vector.memset(warm_w[:].bitcast(U16), 0)
    nc.vector.memset(warmz[:].bitcast(U16), 0)
    nc.vector.memset(vext[:], 1.0)
    nc.vector.memset(ones9[:], 1.0)

    # ---------------- DMA loads --------------------------------------
    # q/k/v weights packed in ONE [128, 24] tensor: a [128, 8] DMA costs
    # 128 tiny descriptors; three of them serialized starved the queues.
    wq3 = const.tile([KC, 3 * D], BF16)
    wqe = wq3[:, 0:D]
    wke = wq3[:, D:2 * D]
    wve = wq3[:, 2 * D:3 * D]
    webe = const.tile([VW, C], BF16)
    sel = const.tile([D, D * NQB], BF16)
    zz_d, wq3_d = dram[8], dram[9]  # noqa: wq/wk/wv packed
    nc.sync.dma_start(wq3[:], wq3_d)
    nc.scalar.dma_start(webe[:], webe_d)
    nc.scalar.dma_start(sel[:], sel_d)
    # x/y slabs interleaved across both queues so slab p of both tensors
    # lands early; zero-fills of qTn/kTn ride along afterwards.
    SLAB = HW // 4
    nc.sync.dma_start(xTe[:, ts(0, SLAB)], xTe_d[:, ts(0, SLAB)])
    nc.scalar.dma_start(yTe[:, ts(0, SLAB)], yTe_d[:, ts(0, SLAB)])
    nc.sync.dma_start(yTe[:, ts(1, SLAB)], yTe_d[:, ts(1, SLAB)])
    nc.scalar.dma_start(xTe[:, ts(1, SLAB)], xTe_d[:, ts(1, SLAB)])
    nc.sync.dma_start(xTe[:, ts(2, SLAB)], xTe_d[:, ts(2, SLAB)])
    nc.scalar.dma_start(yTe[:, ts(2, SLAB)], yTe_d[:, ts(2, SLAB)])
    nc.sync.dma_start(yTe[:, ts(3, SLAB)], yTe_d[:, ts(3, SLAB)])
    nc.scalar.dma_start(xTe[:, ts(3, SLAB)], xTe_d[:, ts(3, SLAB)])
    nc.sync.dma_start(kTn[:], zz_d)
    nc.scalar.dma_start(qTn[:], zz_d)

    # sqrt table preload (ScalarE, overlaps DMA wire time)
    nc.vector.memset(warm[:], 1.0)
    nc.scalar.sqrt(warm[:], warm[:])

    # ---------------- PE warmup (HAM to 8/8 before projections) ---------
    wps = ps_o.tile([KC, QB], F32, tag="pso", name="warmups")
    for i in range(NWARM):
        nc.tensor.matmul(wps[0:16, :], warm_w[:], warmz[:], start=True,
                         stop=True)

    # ---------------- projections (pipelined PSUM pairs at p0) ----------
    # pair p covers q/k blocks 2p,2p+1 -> [D, 2*QB] spanning 2 PSUM banks.
    # q pairs consumed by ACT (copy bf16 + square), k pairs by DVE.
    qps = [None] * 4
    kps = [None] * 4
    for p in range(4):
        qps[p] = ps_s.tile([D, 2 * QB], F32, tag="pss", name=f"qp{p}")
        for u in range(2):
            nc.tensor.matmul(qps[p][:, ts(u, QB)], wqe,
                             xTe[:, ts(2 * p + u, QB)], start=True, stop=True)
        kps[p] = ps_s.tile([D, 2 * QB], F32, tag="pss", name=f"kp{p}")
        for u in range(2):
            nc.tensor.matmul(kps[p][:, ts(u, QB)], wke,
                             yTe[:, ts(2 * p + u, QB)], start=True, stop=True)
        if p < 2:
            # keeper matmuls: bridge DMA waits so HAM never re-throttles
            for _ in range(5 - 2 * p):
                nc.tensor.matmul(wps[0:16, :], warm_w[:], warmz[:],
                                 start=True, stop=True)
    for p in range(4):
        sl = slice(2 * p * QB, (2 * p + 2) * QB)
        nc.scalar.copy(qT[:, sl], qps[p][:])
        nc.vector.tensor_copy(kT[:, sl], kps[p][:])
        nc.vector.tensor_mul(sqk[:, sl], kT[:, sl], kT[:, sl])
        nc.vector.tensor_mul(sqq[:, sl], qT[:, sl], qT[:, sl])

    # ---------------- v prep (row layout into ones-filled vext) ---------
    vps = ps_o.tile([KC, QB], F32, tag="pso", name="vprep")
    for c in range(NKC):
        nc.tensor.matmul(vps[:, c * D:(c + 1) * D], yTe[:, ts(c, KC)],
                         wve, start=True, stop=True)
    # one strided copy for all 32 chunks; ones column FIRST per chunk so
    # the softmax denominator lands at partition 0 of the mm2 accumulator
    vdst = vext[:].rearrange("p (c v) -> p c v", v=VW)[:, :, 1:VW]
    vsrc = vps[:, 0:NKC * D].rearrange("p (c v) -> p c v", v=D)
    nc.vector.tensor_copy(vdst, vsrc)

    # ---------------- inverse norms -------------------------------------
    # selector matmuls K=8: one-hot col j sums squares into out row j.
    # q and k selectors get separate banks: the whole chain must stay at
    # partitions 0-7 (the flattening rep DMA misreads offset-32 sources).
    sps = ps_o.tile([D, QB], F32, tag="pso", name="selps")
    for j in range(NQB):
        nc.tensor.matmul(sps[:], sel[:, ts(j, D)], sqq[:, ts(j, QB)],
                         start=(j == 0), stop=(j == NQB - 1))
    sps_k = ps_o.tile([D, QB], F32, tag="pso", name="selpsk")
    for j in range(NQB):
        nc.tensor.matmul(sps_k[:], sel[:, ts(j, D)], sqk[:, ts(j, QB)],
                         start=(j == 0), stop=(j == NQB - 1))
    nc.scalar.sqrt(sa_q[:], sps[:])
    nc.scalar.sqrt(sa_k[:], sps_k[:])
    # preload the exp table now; reading sa_k pins this AFTER the sqrts so
    # the scheduler cannot hoist it (each hoist costs 2 extra table loads)
    nc.scalar.activation(warm[:], sa_k[0:1, 0:1],
                         mybir.ActivationFunctionType.Exp)
    nc.vector.reciprocal_approx_accurate(inv_kf[:], sa_k[:], scr_k[:])
    nc.vector.tensor_copy(inv_k[:], inv_kf[:])
    nc.vector.reciprocal_approx_accurate(inv_qf[:], sa_q[:], scr_q[:])
    nc.vector.tensor_copy(inv_q[:], inv_qf[:])

    # block 0's inv rows via K=1 PE broadcasts (latency ~1us); the other
    # blocks via row DMAs (k rows first on BOTH queues: the exp stream
    # consumes k blocks at ~2us per block)
    rk0 = ps_o.tile([D, QB], F32, tag="pso", name="rk0")
    nc.tensor.matmul(rk0[:], ones9[:, 0:D], inv_k[0:1, 0:QB], start=True,
                     stop=True)
    rq0 = ps_o.tile([D, QB], F32, tag="pso", name="rq0")
    nc.tensor.matmul(rq0[:], ones9[:, 0:D], inv_q[0:1, 0:QB], start=True,
                     stop=True)
    for p in range(D):
        (nc.sync if p % 2 == 0 else nc.scalar).dma_start(
            rep_k[p:p + 1, :], inv_k[:])
    for p in range(D):
        (nc.sync if p % 2 == 0 else nc.scalar).dma_start(
            rep_q[p:p + 1, :], inv_q[:])

    # normalize muls, per 512-block, consumption-ordered; block 0 reads
    # the PSUM broadcasts, the rest the bf16 SBUF replication (2x mode)
    def _qmul(j):
        nc.vector.tensor_mul(qTn[0:D, ts(j, QB)], qT[:, ts(j, QB)],
                             rep_q[:, ts(j, QB)])

    def _kmul(j):
        nc.vector.tensor_mul(kTn[0:D, ts(j, QB)], kT[:, ts(j, QB)],
                             rep_k[:, ts(j, QB)])

    nc.vector.tensor_mul(kTn[0:D, 0:QB], kT[:, 0:QB], rk0[:])
    nc.vector.tensor_mul(qTn[0:D, 0:QB], qT[:, 0:QB], rq0[:])
    for j in range(1, NQB):
        _kmul(j)
    for j in range(1, NQB):
        _qmul(j)

    # ---------------- main attention loop -------------------------------
    # oTe rows 0-7 numerator, row 8 softmax denominator. Flattened
    # (q-block, group) sequence with one group of mm1 lookahead.
    seq = []
    for j in range(NQB):
        c = 0
        for g in GROUPS:
            seq.append((j, c, g))
            c += g
    blk_start = {j: next(i for i, s in enumerate(seq) if s[0] == j)
                 for j in range(NQB)}
    pos = [None] * NQB
    pss = [None] * len(seq)

    def mm1(i):
        j, c, g = seq[i]
        ps = ps_s.tile([KC, GMAX * QB], F32, tag="pss", name=f"pss{i}")
        pss[i] = ps
        for u in range(g):
            nc.tensor.matmul(ps[:, ts(u, QB)], kTn[:, ts(c + u, KC)],
                             qTn[:, ts(j, QB)], start=True, stop=True)

    es_t = [None] * len(seq)

    def mm2(i):
        j, c, g = seq[i]
        if pos[j] is None:
            pos[j] = ps_o.tile([KC, QB], F32, tag="pso", name=f"po{j}")
        es = es_t[i]
        for u in range(g):
            cc = c + u
            nc.tensor.matmul(pos[j][0:VW, :], vext[:, cc * VW:(cc + 1) * VW],
                             es[:, ts(u, QB)],
                             start=(cc == 0), stop=(cc == NKC - 1))
        es_t[i] = None
        if c + g == NKC:
            epi_a(j)

    # per-block epilogue, staged under the next block's groups; everything
    # lives at partitions 0-8 / 64-127 of the retired pos bank. The den
    # sits at row 0 (ones column first in vext), so the reciprocal reads
    # the PSUM accumulator directly -- no repack DMA.
    def epi_a(j):      # right after last mm2 of block j
        nc.vector.reciprocal_approx_accurate(invd_f[:], pos[j][0:1, :],
                                             dscr[:])
        nc.vector.tensor_copy(invd[:], invd_f[:])
        nc.vector.tensor_copy(oTe[:, ts(j, QB)], pos[j][0:VW, :])

    def epi_b(j):      # PE K=1 broadcast of 1/den into pos[j] rows 0-8
        nc.tensor.matmul(pos[j][0:VW, :], ones9[:, 0:VW], invd[:],
                         start=True, stop=True)

    def epi_c(j):      # stage mul + K=9 output projection + copy + DMA out
        nc.vector.tensor_mul(stage[:, ts(j, QB)], oTe[:, ts(j, QB)],
                             pos[j][0:VW, :])
        nc.tensor.matmul(pos[j][64:64 + C, :], webe[:],
                         stage[:, ts(j, QB)], start=True, stop=True)
        nc.vector.tensor_copy(resT[64:64 + C, ts(j, QB)],
                              pos[j][64:64 + C, :])
        nc.sync.dma_start(out_d[:, ts(j, QB)], resT[64:64 + C, ts(j, QB)])

    pending = {}

    # software pipeline with mm2 deferred one group: while exp(i) runs on
    # ScalarE, the PE executes mm1(i+1) then mm2(i-1) -- so exp(i+1) never
    # sits behind an mm2 that itself waits on exp(i) (head-of-line stall)
    mm1(0)
    for i, (j, c, g) in enumerate(seq):
        for fn in pending.pop(i, ()):
            fn()
        if i + 1 < len(seq):
            mm1(i + 1)
        es = expp.tile([KC, GMAX * QB], BF16, tag="es")
        es_t[i] = es
        nc.scalar.activation(es[:, 0:g * QB], pss[i][:, 0:g * QB],
                             mybir.ActivationFunctionType.Exp)
        pss[i] = None
        if i > 0:
            mm2(i - 1)
        if c + g == NKC and j + 1 < NQB:
            s = blk_start[j + 1]
            pending.setdefault(s + 3, []).append(lambda j=j: epi_b(j))
            pending.setdefault(s + 5, []).append(lambda j=j: epi_c(j))
    mm2(len(seq) - 1)
    epi_b(NQB - 1)
    epi_c(NQB - 1)

    if DEBUG:
        dbg = dram[10]
        nc.sync.dma_start(dbg["qTn"], qTn[0:D, :])
        nc.sync.dma_start(dbg["kTn"], kTn[0:D, :])
        nc.sync.dma_start(dbg["vext"], vext[:])
        nc.sync.dma_start(dbg["oTe"], oTe[:])
        nc.sync.dma_start(dbg["repq"], rep_q[:])
        nc.sync.dma_start(dbg["repk"], rep_k[:])
        nc.sync.dma_start(dbg["sqq"], sqq[:])
        nc.sync.dma_start(dbg["qT"], qT[:])
        nc.sync.dma_start(dbg["kT"], kT[:])


def _build():
    global _BUILT
    if _BUILT is not None:
        return _BUILT
    nc = bacc.Bacc("TRN2", target_bir_lowering=False, debug=False, num_devices=H)
    xTe_d = nc.dram_tensor("xTe", [KC, HW], BF16, kind="ExternalInput").ap()
    yTe_d = nc.dram_tensor("yTe", [KC, HW], BF16, kind="ExternalInput").ap()
    wq3_d = nc.dram_tensor("wq3", [KC, 3 * D], BF16, kind="ExternalInput").ap()
    webe_d = nc.dram_tensor("webe", [VW, C], BF16, kind="ExternalInput").ap()
    sel_d = nc.dram_tensor("sel", [D, D * NQB], BF16, kind="ExternalInput").ap()
    zz_d = nc.dram_tensor("zz", [KC, HW], BF16, kind="ExternalInput").ap()
    out_d = nc.dram_tensor("resT", [C, HW], F32, kind="ExternalOutput").ap()
    dbg = None
    if DEBUG:
        dbg = {
            "qTn": nc.dram_tensor("d_qTn", [D, HW], BF16, kind="ExternalOutput").ap(),
            "kTn": nc.dram_tensor("d_kTn", [D, HW], BF16, kind="ExternalOutput").ap(),
            "vext": nc.dram_tensor("d_vext", [KC, VW * NKC], BF16, kind="ExternalOutput").ap(),
            "oTe": nc.dram_tensor("d_oTe", [VW, HW], F32, kind="ExternalOutput").ap(),
            "repq": nc.dram_tensor("d_repq", [D, HW], BF16, kind="ExternalOutput").ap(),
            "repk": nc.dram_tensor("d_repk", [D, HW], BF16, kind="ExternalOutput").ap(),
            "sqq": nc.dram_tensor("d_sqq", [D, HW], BF16, kind="ExternalOutput").ap(),
            "qT": nc.dram_tensor("d_qT", [D, HW], BF16, kind="ExternalOutput").ap(),
            "kT": nc.dram_tensor("d_kT", [D, HW], BF16, kind="ExternalOutput").ap(),
        }
    with tile.TileContext(nc) as tc, ExitStack() as ctx:
        _body(ctx, tc, (xTe_d, yTe_d, None, None, None, webe_d, sel_d,
                        out_d[:], zz_d, wq3_d, dbg))
    nc.compile()
    _BUILT = nc
    return nc


def make_in_maps(x, y, Wq, bq, Wkv, bkv, We, be):
    import ml_dtypes
    bf16 = ml_dtypes.bfloat16
    x, y, Wq, bq, Wkv, bkv, We, be = (
        np.asarray(a, np.float32) for a in (x, y, Wq, bq, Wkv, bkv, We, be))
    ones = np.ones((1, HW), np.float32)
    zrows = np.zeros((KC - CE, HW), np.float32)
    xTe = np.ascontiguousarray(np.vstack([x[0].T, ones, zrows])).astype(bf16)
    yTe = np.ascontiguousarray(np.vstack([y[0].T, ones, zrows])).astype(bf16)
    zz = np.zeros((KC, HW), np.float32).astype(bf16)
    sel = np.zeros((D, D * NQB), np.float32)
    for j in range(NQB):
        sel[0:D, D * j + j] = 1.0
    sel = sel.astype(bf16)
    zpad = np.zeros((KC - CE, D), np.float32)
    in_maps = []
    for h in range(H):
        sl = slice(h * D, (h + 1) * D)
        slv = slice(C + h * D, C + (h + 1) * D)
        wqe = np.vstack([Wq[:, sl], bq[None, sl], zpad])
        wke = np.vstack([Wkv[:, sl], bkv[None, sl], zpad])
        wve = np.vstack([Wkv[:, slv], bkv[None, slv], zpad])
        in_maps.append({
            "xTe": xTe,
            "yTe": yTe,
            "wq3": np.ascontiguousarray(
                np.hstack([wqe, wke, wve])).astype(bf16),
            # bias row FIRST: stage row 0 is the 1.0 (den*inv) row
            "webe": np.ascontiguousarray(
                np.vstack([be[None, :] / H, We[sl, :]])).astype(bf16),
            "sel": sel,
            "zz": zz,
        })
    return in_maps


def kernel(x, y, Wq, bq, Wkv, bkv, We, be):
    global LAST_RESULTS
    nc = _build()
    in_maps = make_in_maps(x, y, Wq, bq, Wkv, bkv, We, be)
    res = run_bass_kernel_spmd(nc, in_maps, core_ids=list(range(H)), trace=TRACE)
    LAST_RESULTS = res
    acc = np.zeros((C, HW), np.float64)
    for r in res.results:
        acc += r["resT"]
    return np.ascontiguousarray(acc.T[None]).astype(np.float32)


# revision 45
# speedup vs baseline: 1.1901x; 1.0150x over previous
"""Trainium2 Bass kernel for cross-attention (cosine-normalized, 8 heads).

Reference computation (full inputs x,y [1,4096,64]):
  q = x@Wq+bq ; k,v = split(y@Wkv+bkv) ; per head (8 heads, dim 8):
  attn = softmax(l2norm(q) @ l2norm(k)^T) ; out = attn@v
  result = concat_heads(out) @ We + be
Sharding: one head per NeuronCore, host sums per-core resT partials.

v2 rewrite of the 217us baseline, targeting the exp floor:
  - Steady state is exp-bound on ScalarE (16.7M exps/core ~ 132us); the
    baseline wasted ~50us of prologue + ~25us tail around it.
  - bf16 everywhere on the PE (the baseline's f32r silently lowered to
    fp32_mode=HIGH at 4 cyc/row; bf16 is 1 cyc/row). Inputs ship bf16.
  - PE warmup matmuls at t=0 so projections run at 2.4 GHz, no gpsimd
    (its memsets + drain gated the old norm chain), DVE memsets instead.
  - Projections go through pipelined 2-bank PSUM pair tiles (pool
    rotation), q-path consumed by ACT (copy+square), k-path by DVE.
  - Selector matmuls at K=8 (no zero-padding of the squares tiles
    needed); sqrt reads selector PSUM directly; exp table preloaded via
    a dummy exp right after the sqrts.
  - Norm replication via row DMAs (sync=q, scalar=k queues), normalize
    muls in bf16 2x mode, ordered q0 / k-pairs first so the main loop
    starts ASAP.
  - Per-block epilogue fully overlapped under the next block's exp
    stream: den row DMA -> recip -> PE K=1 broadcast into the retired
    pos PSUM bank -> stage mul -> K=9 output projection at p64-127 of
    the same bank -> DVE copy -> sync DMA out. ScalarE does nothing but
    exp during the main loop.
"""

import sys

import numpy as np

for _p in ("/opt/trn_rl_repo",):
    if _p not in sys.path:
        sys.path.insert(0, _p)

from contextlib import ExitStack

import concourse.bass as bass
import concourse.tile as tile
from concourse import bacc, mybir
from concourse.bass import ts
from concourse.bass_utils import run_bass_kernel_spmd

F32 = mybir.dt.float32
BF16 = mybir.dt.bfloat16

HW = 4096          # sequence length
C = 64             # model dim
H = 8              # heads
D = 8              # head dim
CE = C + 1         # +ones row for bias folding
QB = 512           # q block
NQB = HW // QB     # 8
KC = 128           # k chunk
NKC = HW // KC     # 32
GROUPS = [3] * 10 + [2]   # k-chunks per exp/ACT group (32 total)
GMAX = max(GROUPS)
VW = D + 1         # v + ones column
NWARM = 12         # PE warmup matmuls

_BUILT = None
TRACE = False
LAST_RESULTS = None
DEBUG = False


def _body(ctx, tc, dram):
    nc = tc.nc
    xTe_d, yTe_d, wqe_d, wke_d, wve_d, webe_d, sel_d, out_d = dram[:8]

    const = ctx.enter_context(tc.tile_pool(name="const", bufs=1))
    expp = ctx.enter_context(tc.tile_pool(name="exps", bufs=4))
    ps_s = ctx.enter_context(tc.tile_pool(name="ps_s", bufs=2, space="PSUM"))
    ps_o = ctx.enter_context(tc.tile_pool(name="ps_o", bufs=2, space="PSUM"))

    # ---------------- SBUF tiles ----------------
    xTe = const.tile([KC, HW], BF16)     # rows 0..64 DMA'd, 65.. zeroed
    yTe = const.tile([KC, HW], BF16)
    qTn = const.tile([KC, HW], BF16)     # normalized q, rows 8.. zero
    kTn = const.tile([KC, HW], BF16)
    vext = const.tile([KC, VW * NKC], BF16)
    qT = const.tile([D, HW], BF16)       # raw q (transposed), bf16
    kT = const.tile([D, HW], BF16)
    sqq = const.tile([D, HW], BF16)      # squares (bf16, selector rhs)
    sqk = const.tile([D, HW], BF16)
    rep_q = const.tile([D, HW], BF16)    # inv norms replicated to D rows
    rep_k = const.tile([D, HW], BF16)
    oTe = const.tile([VW, HW], F32)      # den row 0 + numerator rows 1-8
    stage = const.tile([VW, HW], BF16)   # normalized, den row == 1.0
    resT = const.tile([KC, HW], F32)     # output staging (rows 64..127)
    invd_f = const.tile([1, QB], F32)
    invd = const.tile([1, QB], BF16)
    dscr = const.tile([1, QB], F32)
    ones9 = const.tile([1, 16], BF16)
    warm_w = const.tile([KC, 16], BF16)
    sa_q = const.tile([D, QB], F32)      # sqrt of sum-squares
    sa_k = const.tile([D, QB], F32)
    inv_qf = const.tile([D, QB], F32)
    inv_kf = const.tile([D, QB], F32)
    inv_q = const.tile([D, QB], BF16)
    inv_k = const.tile([D, QB], BF16)
    scr_q = const.tile([D, QB], F32)
    scr_k = const.tile([D, QB], F32)
    warm = const.tile([1, 1], F32)

    # ---------------- t=0: small DVE memsets only -----------------------
    # (big zero-fills come from DRAM: a [128,4096] DVE memset is 3.5us
    # FD-serial; host ships padded xTe/yTe and a zeros tensor instead)
    U16 = mybir.dt.uint16
    warmz = const.tile([KC, QB], BF16)
    nc.vector.memset(warm_w[:].bitcast(U16), 0)
    nc.vector.memset(warmz[:].bitcast(U16), 0)
    nc.vector.memset(vext[:], 1.0)
    nc.vector.memset(ones9[:], 1.0)

    # ---------------- DMA loads --------------------------------------
    # q/k/v weights packed in ONE [128, 24] tensor: a [128, 8] DMA costs
    # 128 tiny descriptors; three of them serialized starved the queues.
    wq3 = const.tile([KC, 3 * D], BF16)
    wqe = wq3[:, 0:D]
    wke = wq3[:, D:2 * D]
    wve = wq3[:, 2 * D:3 * D]
    webe = const.tile([VW, C], BF16)
    sel = const.tile([D, D * NQB], BF16)
    zz_d, wq3_d = dram[8], dram[9]  # noqa: wq/wk/wv packed
    # keep the scalar (ACT) queue nearly free: every DMA issue there
    # steals ~0.6-1.4us from the engine that must start the q copies
    SLAB = HW // 4
    nc.sync.dma_start(wq3[:], wq3_d)
    nc.sync.dma_start(xTe[:, ts(0, SLAB)], xTe_d[:, ts(0, SLAB)])
    nc.scalar.dma_start(yTe[:, ts(0, SLAB)], yTe_d[:, ts(0, SLAB)])
    nc.sync.dma_start(yTe[:, ts(1, SLAB)], yTe_d[:, ts(1, SLAB)])
    nc.scalar.dma_start(xTe[:, ts(1, SLAB)], xTe_d[:, ts(1, SLAB)])
    nc.sync.dma_start(xTe[:, ts(2, SLAB)], xTe_d[:, ts(2, SLAB)])
    nc.sync.dma_start(yTe[:, ts(2, SLAB)], yTe_d[:, ts(2, SLAB)])
    nc.sync.dma_start(yTe[:, ts(3, SLAB)], yTe_d[:, ts(3, SLAB)])
    nc.sync.dma_start(xTe[:, ts(3, SLAB)], xTe_d[:, ts(3, SLAB)])
    nc.scalar.dma_start(qTn[:], zz_d)
    nc.sync.dma_start(kTn[:], zz_d)
    nc.sync.dma_start(webe[:], webe_d)
    nc.sync.dma_start(sel[:], sel_d)

    # sqrt table preload (ScalarE, overlaps DMA wire time)
    nc.vector.memset(warm[:], 1.0)
    nc.scalar.sqrt(warm[:], warm[:])

    # ---------------- PE warmup (HAM to 8/8 before projections) ---------
    wps = ps_o.tile([KC, QB], F32, tag="pso", name="warmups")
    for i in range(NWARM):
        nc.tensor.matmul(wps[0:16, :], warm_w[:], warmz[:], start=True,
                         stop=True)

    # ---------------- projections (pipelined PSUM pairs at p0) ----------
    # pair p covers q/k blocks 2p,2p+1 -> [D, 2*QB] spanning 2 PSUM banks.
    # q pairs consumed by ACT (copy bf16 + square), k pairs by DVE.
    qps = [None] * 4
    kps = [None] * 4
    for p in range(4):
        qps[p] = ps_s.tile([D, 2 * QB], F32, tag="pss", name=f"qp{p}")
        for u in range(2):
            nc.tensor.matmul(qps[p][:, ts(u, QB)], wqe,
                             xTe[:, ts(2 * p + u, QB)], start=True, stop=True)
        kps[p] = ps_s.tile([D, 2 * QB], F32, tag="pss", name=f"kp{p}")
        for u in range(2):
            nc.tensor.matmul(kps[p][:, ts(u, QB)], wke,
                             yTe[:, ts(2 * p + u, QB)], start=True, stop=True)
        if p < 2:
            # keeper matmuls: bridge DMA waits so HAM never re-throttles
            for _ in range(5 - 2 * p):
                nc.tensor.matmul(wps[0:16, :], warm_w[:], warmz[:],
                                 start=True, stop=True)
    for p in range(4):
        sl = slice(2 * p * QB, (2 * p + 2) * QB)
        nc.scalar.copy(qT[:, sl], qps[p][:])
        nc.vector.tensor_copy(kT[:, sl], kps[p][:])
        nc.vector.tensor_mul(sqk[:, sl], kT[:, sl], kT[:, sl])
        nc.vector.tensor_mul(sqq[:, sl], qT[:, sl], qT[:, sl])

    # ---------------- v prep (row layout into ones-filled vext) ---------
    vps = ps_o.tile([KC, QB], F32, tag="pso", name="vprep")
    for c in range(NKC):
        nc.tensor.matmul(vps[:, c * D:(c + 1) * D], yTe[:, ts(c, KC)],
                         wve, start=True, stop=True)
    # one strided copy for all 32 chunks; ones column FIRST per chunk so
    # the softmax denominator lands at partition 0 of the mm2 accumulator
    vdst = vext[:].rearrange("p (c v) -> p c v", v=VW)[:, :, 1:VW]
    vsrc = vps[:, 0:NKC * D].rearrange("p (c v) -> p c v", v=D)
    nc.vector.tensor_copy(vdst, vsrc)

    # ---------------- inverse norms -------------------------------------
    # selector matmuls K=8: one-hot col j sums squares into out row j.
    # k selectors first (the k squares are ready well before the q ones,
    # which sit behind the ACT copies); keepers bridge the wait for sqq.
    sps_k = ps_o.tile([D, QB], F32, tag="pso", name="selpsk")
    for j in range(NQB):
        nc.tensor.matmul(sps_k[:], sel[:, ts(j, D)], sqk[:, ts(j, QB)],
                         start=(j == 0), stop=(j == NQB - 1))
    for _ in range(4):
        nc.tensor.matmul(wps[0:16, :], warm_w[:], warmz[:], start=True,
                         stop=True)
    sps = ps_o.tile([D, QB], F32, tag="pso", name="selps")
    for j in range(NQB):
        nc.tensor.matmul(sps[:], sel[:, ts(j, D)], sqq[:, ts(j, QB)],
                         start=(j == 0), stop=(j == NQB - 1))
    nc.scalar.sqrt(sa_k[:], sps_k[:])
    nc.scalar.sqrt(sa_q[:], sps[:])
    # preload the exp table now; reading sa_q pins this AFTER the sqrts so
    # the scheduler cannot hoist it (each hoist costs 2 extra table loads)
    nc.scalar.activation(warm[:], sa_q[0:1, 0:1],
                         mybir.ActivationFunctionType.Exp)
    nc.vector.reciprocal_approx_accurate(inv_kf[:], sa_k[:], scr_k[:])
    nc.vector.tensor_copy(inv_k[:], inv_kf[:])
    nc.vector.reciprocal_approx_accurate(inv_qf[:], sa_q[:], scr_q[:])
    nc.vector.tensor_copy(inv_q[:], inv_qf[:])

    # block 0's inv rows via K=1 PE broadcasts (latency ~1us); the other
    # blocks via row DMAs (k rows first on BOTH queues: the exp stream
    # consumes k blocks at ~2us per block)
    rk0 = ps_o.tile([D, QB], F32, tag="pso", name="rk0")
    nc.tensor.matmul(rk0[:], ones9[:, 0:D], inv_k[0:1, 0:QB], start=True,
                     stop=True)
    rq0 = ps_o.tile([D, QB], F32, tag="pso", name="rq0")
    nc.tensor.matmul(rq0[:], ones9[:, 0:D], inv_q[0:1, 0:QB], start=True,
                     stop=True)
    for p in range(D):
        (nc.sync if p % 2 == 0 else nc.scalar).dma_start(
            rep_k[p:p + 1, :], inv_k[:])
    for p in range(D):
        (nc.sync if p % 2 == 0 else nc.scalar).dma_start(
            rep_q[p:p + 1, :], inv_q[:])

    # normalize muls, per 512-block, consumption-ordered; block 0 reads
    # the PSUM broadcasts, the rest the bf16 SBUF replication (2x mode)
    def _qmul(j):
        nc.vector.tensor_mul(qTn[0:D, ts(j, QB)], qT[:, ts(j, QB)],
                             rep_q[:, ts(j, QB)])

    def _kmul(j):
        nc.vector.tensor_mul(kTn[0:D, ts(j, QB)], kT[:, ts(j, QB)],
                             rep_k[:, ts(j, QB)])

    nc.vector.tensor_mul(kTn[0:D, 0:QB], kT[:, 0:QB], rk0[:])
    nc.vector.tensor_mul(qTn[0:D, 0:QB], qT[:, 0:QB], rq0[:])
    for j in range(1, NQB):
        _kmul(j)
    for j in range(1, NQB):
        _qmul(j)

    # ---------------- main attention loop -------------------------------
    # oTe rows 0-7 numerator, row 8 softmax denominator. Flattened
    # (q-block, group) sequence with one group of mm1 lookahead.
    seq = []
    for j in range(NQB):
        c = 0
        for g in GROUPS:
            seq.append((j, c, g))
            c += g
    blk_start = {j: next(i for i, s in enumerate(seq) if s[0] == j)
                 for j in range(NQB)}
    pos = [None] * NQB
    pss = [None] * len(seq)

    def mm1(i):
        j, c, g = seq[i]
        ps = ps_s.tile([KC, GMAX * QB], F32, tag="pss", name=f"pss{i}")
        pss[i] = ps
        for u in range(g):
            nc.tensor.matmul(ps[:, ts(u, QB)], kTn[:, ts(c + u, KC)],
                             qTn[:, ts(j, QB)], start=True, stop=True)

    es_t = [None] * len(seq)

    def mm2(i):
        j, c, g = seq[i]
        if pos[j] is None:
            pos[j] = ps_o.tile([KC, QB], F32, tag="pso", name=f"po{j}")
        es = es_t[i]
        for u in range(g):
            cc = c + u
            nc.tensor.matmul(pos[j][0:VW, :], vext[:, cc * VW:(cc + 1) * VW],
                             es[:, ts(u, QB)],
                             start=(cc == 0), stop=(cc == NKC - 1))
        es_t[i] = None
        if c + g == NKC:
            epi_a(j)

    # per-block epilogue, staged under the next block's groups; everything
    # lives at partitions 0-8 / 64-127 of the retired pos bank. The den
    # sits at row 0 (ones column first in vext), so the reciprocal reads
    # the PSUM accumulator directly -- no repack DMA.
    def epi_a(j):      # right after last mm2 of block j
        nc.vector.reciprocal_approx_accurate(invd_f[:], pos[j][0:1, :],
                                             dscr[:])
        nc.vector.tensor_copy(invd[:], invd_f[:])
        nc.vector.tensor_copy(oTe[:, ts(j, QB)], pos[j][0:VW, :])

    def epi_b(j):      # PE K=1 broadcast of 1/den into pos[j] rows 0-8
        nc.tensor.matmul(pos[j][0:VW, :], ones9[:, 0:VW], invd[:],
                         start=True, stop=True)

    def epi_c(j):      # stage mul + K=9 output projection + copy + DMA out
        nc.vector.tensor_mul(stage[:, ts(j, QB)], oTe[:, ts(j, QB)],
                             pos[j][0:VW, :])
        nc.tensor.matmul(pos[j][64:64 + C, :], webe[:],
                         stage[:, ts(j, QB)], start=True, stop=True)
        nc.vector.tensor_copy(resT[64:64 + C, ts(j, QB)],
                              pos[j][64:64 + C, :])
        nc.sync.dma_start(out_d[:, ts(j, QB)], resT[64:64 + C, ts(j, QB)])

    pending = {}

    # software pipeline with mm2 deferred one group: while exp(i) runs on
    # ScalarE, the PE executes mm1(i+1) then mm2(i-1) -- so exp(i+1) never
    # sits behind an mm2 that itself waits on exp(i) (head-of-line stall)
    mm1(0)
    for i, (j, c, g) in enumerate(seq):
        for fn in pending.pop(i, ()):
            fn()
        if i + 1 < len(seq):
            mm1(i + 1)
        es = expp.tile([KC, GMAX * QB], BF16, tag="es")
        es_t[i] = es
        nc.scalar.activation(es[:, 0:g * QB], pss[i][:, 0:g * QB],
                             mybir.ActivationFunctionType.Exp)
        pss[i] = None
        if i > 0:
            mm2(i - 1)
        if c + g == NKC and j + 1 < NQB:
            s = blk_start[j + 1]
            pending.setdefault(s + 3, []).append(lambda j=j: epi_b(j))
            pending.setdefault(s + 5, []).append(lambda j=j: epi_c(j))
    mm2(len(seq) - 1)
    epi_b(NQB - 1)
    epi_c(NQB - 1)

    if DEBUG:
        dbg = dram[10]
        nc.sync.dma_start(dbg["qTn"], qTn[0:D, :])
        nc.sync.dma_start(dbg["kTn"], kTn[0:D, :])
        nc.sync.dma_start(dbg["vext"], vext[:])
        nc.sync.dma_start(dbg["oTe"], oTe[:])
        nc.sync.dma_start(dbg["repq"], rep_q[:])
        nc.sync.dma_start(dbg["repk"], rep_k[:])
        nc.sync.dma_start(dbg["sqq"], sqq[:])
        nc.sync.dma_start(dbg["qT"], qT[:])
        nc.sync.dma_start(dbg["kT"], kT[:])


def _build():
    global _BUILT
    if _BUILT is not None:
        return _BUILT
    nc = bacc.Bacc("TRN2", target_bir_lowering=False, debug=False, num_devices=H)
    xTe_d = nc.dram_tensor("xTe", [KC, HW], BF16, kind="ExternalInput").ap()
    yTe_d = nc.dram_tensor("yTe", [KC, HW], BF16, kind="ExternalInput").ap()
    wq3_d = nc.dram_tensor("wq3", [KC, 3 * D], BF16, kind="ExternalInput").ap()
    webe_d = nc.dram_tensor("webe", [VW, C], BF16, kind="ExternalInput").ap()
    sel_d = nc.dram_tensor("sel", [D, D * NQB], BF16, kind="ExternalInput").ap()
    zz_d = nc.dram_tensor("zz", [KC, HW], BF16, kind="ExternalInput").ap()
    out_d = nc.dram_tensor("resT", [C, HW], F32, kind="ExternalOutput").ap()
    dbg = None
    if DEBUG:
        dbg = {
            "qTn": nc.dram_tensor("d_qTn", [D, HW], BF16, kind="ExternalOutput").ap(),
            "kTn": nc.dram_tensor("d_kTn", [D, HW], BF16, kind="ExternalOutput").ap(),
            "vext": nc.dram_tensor("d_vext", [KC, VW * NKC], BF16, kind="ExternalOutput").ap(),
            "oTe": nc.dram_tensor("d_oTe", [VW, HW], F32, kind="ExternalOutput").ap(),
            "repq": nc.dram_tensor("d_repq", [D, HW], BF16, kind="ExternalOutput").ap(),
            "repk": nc.dram_tensor("d_repk", [D, HW], BF16, kind="ExternalOutput").ap(),
            "sqq": nc.dram_tensor("d_sqq", [D, HW], BF16, kind="ExternalOutput").ap(),
            "qT": nc.dram_tensor("d_qT", [D, HW], BF16, kind="ExternalOutput").ap(),
            "kT": nc.dram_tensor("d_kT", [D, HW], BF16, kind="ExternalOutput").ap(),
        }
    with tile.TileContext(nc) as tc, ExitStack() as ctx:
        _body(ctx, tc, (xTe_d, yTe_d, None, None, None, webe_d, sel_d,
                        out_d[:], zz_d, wq3_d, dbg))
    nc.compile()
    _BUILT = nc
    return nc


def make_in_maps(x, y, Wq, bq, Wkv, bkv, We, be):
    import ml_dtypes
    bf16 = ml_dtypes.bfloat16
    x, y, Wq, bq, Wkv, bkv, We, be = (
        np.asarray(a, np.float32) for a in (x, y, Wq, bq, Wkv, bkv, We, be))
    ones = np.ones((1, HW), np.float32)
    zrows = np.zeros((KC - CE, HW), np.float32)
    xTe = np.ascontiguousarray(np.vstack([x[0].T, ones, zrows])).astype(bf16)
    yTe = np.ascontiguousarray(np.vstack([y[0].T, ones, zrows])).astype(bf16)
    zz = np.zeros((KC, HW), np.float32).astype(bf16)
    sel = np.zeros((D, D * NQB), np.float32)
    for j in range(NQB):
        sel[0:D, D * j + j] = 1.0
    sel = sel.astype(bf16)
    zpad = np.zeros((KC - CE, D), np.float32)
    in_maps = []
    for h in range(H):
        sl = slice(h * D, (h + 1) * D)
        slv = slice(C + h * D, C + (h + 1) * D)
        wqe = np.vstack([Wq[:, sl], bq[None, sl], zpad])
        wke = np.vstack([Wkv[:, sl], bkv[None, sl], zpad])
        wve = np.vstack([Wkv[:, slv], bkv[None, slv], zpad])
        in_maps.append({
            "xTe": xTe,
            "yTe": yTe,
            "wq3": np.ascontiguousarray(
                np.hstack([wqe, wke, wve])).astype(bf16),
            # bias row FIRST: stage row 0 is the 1.0 (den*inv) row
            "webe": np.ascontiguousarray(
                np.vstack([be[None, :] / H, We[sl, :]])).astype(bf16),
            "sel": sel,
            "zz": zz,
        })
    return in_maps


def kernel(x, y, Wq, bq, Wkv, bkv, We, be):
    global LAST_RESULTS
    nc = _build()
    in_maps = make_in_maps(x, y, Wq, bq, Wkv, bkv, We, be)
    res = run_bass_kernel_spmd(nc, in_maps, core_ids=list(range(H)), trace=TRACE)
    LAST_RESULTS = res
    acc = np.zeros((C, HW), np.float64)
    for r in res.results:
        acc += r["resT"]
    return np.ascontiguousarray(acc.T[None]).astype(np.float32)
